# revision 50
# baseline (speedup 1.0000x reference)
"""BNN-MNIST forward pass as a hand-written Bass/Tile kernel, data-parallel
across 8 TRN2 NeuronCores (batch 1024 -> 128 images per core).

Numerical scheme (everything except conv1 is EXACT vs the fp32 reference):
  - conv1: weights are binarized (+-1, exact in bf16); x is split exactly into
    3 bf16 planes (x = hi + mid + lo). The 3x3 conv over 1 input channel is a
    single matmul with contraction = (parity 2, level 3, tap 9) = 54 rows and
    lhs free = (parity 2, out-channel 64) = 128 (block-diagonal weights), so
    two images are computed per streamed column. Products are exact; only the
    PE fp32 accumulation order differs from the CPU reference (ulp-level).
    The 9 tap-shifted copies of each (parity, level) plane are produced
    ON-CHIP: each 32-pair half of the compact [6, 64, 904] DRAM image is
    loaded once with big contiguous descriptors, then 9 SBUF->SBUF
    replication DMAs apply the tap shifts (partition-strided dst) --
    avoiding 5.8 MB of re-reads through the slow HBM DMA path. conv1
    matmuls write even/odd rows to split PSUM halves via a permuted out-AP.
  - sign1 (bn+clip+binarize folded to sign(h + t1)): ACT Sign, +-1 bf16;
    maxpool1 on DVE (vertical max reads the two contiguous row-halves,
    horizontal max strided) into fp8 a1pad; conv1 and conv2 are
    emission-interleaved so the PE alternates between them.
  - conv2: +-1 weights x +-1 activations in fp8, 9 taps accumulate in PSUM
    (exact integers). Contraction = (parity 2, in-channel 64) block-diag.
  - sign2: step(p - m2) per element on DVE (0/1, exact integer compare),
    maxpool2 on DVE.
  - fc1: 0/1 activations vs +-1 weights in fp8, exact integer PSUM; the 0/1
    correction and bn3 threshold fold into thr3[o] = (m3' + K1[o])/2, which
    is subtracted inside the PSUM accumulation by a K=1 matmul
    (ones^T x -thr3) so the activation is a compare against zero. The
    a2 -> k-major transpose is done with SBUF->SBUF partition-scatter DMA
    waves (no DRAM round trip); fc1 matmuls read the resident tile directly.
  - fc2: 0/1 activations, exact integer result J; a3 chunks are transposed on
    the PE (identity-matmul transpose mode) instead of DMA-transposes. Host
    computes the exact affine fixup out = (2J - sum(wfc2b) + bfc2) * scale.
  - A burst of warmup matmuls on a zeroed tile keeps the PE busy from t=0 so
    the HAM clock-gate lifts to 2.4 GHz before conv1's first real matmul.
"""

import functools
import numpy as np
import ml_dtypes

import concourse.bass as bass
import concourse.tile as tile
from concourse import bacc, mybir
from concourse.ap import AP
from concourse.bass_utils import run_bass_kernel_spmd

F32 = mybir.dt.float32
BF16 = mybir.dt.bfloat16
F8 = mybir.dt.float8e4

NP_BF16 = ml_dtypes.bfloat16
NP_F8 = ml_dtypes.float8_e4m3

EPS = 1e-5
N_CORES = 8
BPC = 128          # images per core
PAIRS = BPC // 2   # image pairs per core
TAPS = [(dy, dx) for dy in range(3) for dx in range(3)]
TAP_PAIRS = [(0, 1), (2, 3), (4, 5), (6, 7)]   # DoubleRow tap pairs; tap 8 single
N_WARMUP = 48      # warmup matmuls (N=512) to lift HAM before conv1


# ---------------------------------------------------------------------------
# Device kernel builder
# ---------------------------------------------------------------------------

def _build_nc(reps=1):
    nc = bacc.Bacc("TRN2", target_bir_lowering=False, debug=False,
                   num_devices=N_CORES)

    xp6 = nc.declare_dram_parameter("xp6", [6, PAIRS, 904], BF16, isOutput=False)
    w1s = nc.declare_dram_parameter("w1s", [54, 128], BF16, isOutput=False)
    w2s = nc.declare_dram_parameter("w2s", [128, 9, 128], F8, isOutput=False)
    wfc1 = nc.declare_dram_parameter("wfc1r", [128, 25, 2048], F8, isOutput=False)
    wfc2 = nc.declare_dram_parameter("wfc2r", [128, 16, 10], BF16, isOutput=False)
    t1v = nc.declare_dram_parameter("t1v", [128, 1], F32, isOutput=False)
    m2v = nc.declare_dram_parameter("m2v", [128, 1], F32, isOutput=False)
    thr3 = nc.declare_dram_parameter("thr3n", [1, 2048], F32, isOutput=False)
    ident = nc.declare_dram_parameter("ident", [128, 128], BF16, isOutput=False)
    outp = nc.declare_dram_parameter("out", [BPC, 10], F32, isOutput=True)

    with tile.TileContext(nc) as tc:
        _body(nc, tc, xp6, w1s, w2s, wfc1, wfc2, t1v, m2v, thr3, ident, outp,
              reps=reps)

    nc.compile()
    return nc


def _body(nc, tc, xp6, w1s, w2s, wfc1, wfc2, t1v, m2v, thr3, ident, outp,
          reps=1):
    from contextlib import ExitStack
    with ExitStack() as ctx:
        consts = ctx.enter_context(tc.tile_pool(name="consts", bufs=1))
        apool = ctx.enter_context(tc.tile_pool(name="acts", bufs=1))
        s1pool = ctx.enter_context(tc.tile_pool(name="s1", bufs=2))
        vpool = ctx.enter_context(tc.tile_pool(name="vt", bufs=2))
        s2pool = ctx.enter_context(tc.tile_pool(name="s2", bufs=3))

        for _rep in range(reps):
            # ------------------------------------------------------------------
            # Warmup: PE matmuls on a zeroed tile from t=0 so the HAM clock
            # gate flips to 8/8 (~3.4us of sustained activity) while the
            # conv1 input replication DMAs are still in flight.
            # ------------------------------------------------------------------
            wz = consts.tile([128, 512], F8, tag="wz")
            nc.vector.memset(wz[:], 0)
            with tc.tile_pool(name=f"wps{_rep}", bufs=1, space="PSUM") as wpsp:
                wps = wpsp.tile([128, 512], F32)
                for _ in range(N_WARMUP):
                    nc.tensor.matmul(wps[:], wz[:, 0:128], wz[:],
                                     start=True, stop=True)

            # w1t (first conv1 matmul) and t1t (first sign) are needed
            # early and are tiny -- load them ahead of the replication waves
            w1t = consts.tile([54, 128], BF16)
            nc.sync.dma_start(w1t[:], w1s[:])
            t1t = consts.tile([128, 1], F32)
            nc.scalar.dma_start(t1t[:], t1v[:])

            # ------------------------------------------------------------------
            # conv1 input: on-chip 9-tap replication. xpt[p = g*27+l*9+t] holds
            # the (g,l) plane shifted by tap t. 18 DMAs (9 taps x 2 pair
            # halves) DRAM->SBUF, partition-strided dst, spread over 4 queues.
            # ------------------------------------------------------------------
            xpool_cm = tc.tile_pool(name=f"xplanes{_rep}", bufs=1)
            xpool = xpool_cm.__enter__()
            xpt = xpool.tile([54, PAIRS, 840], BF16)
            xv54 = xpt[:].rearrange("(gl t) pair e -> gl t pair e", t=9)
            a1pad = apool.tile([128, PAIRS, 256], F8)
            # zero only the padding ring of each 16x16 pair-plane
            a1r = a1pad[:].rearrange("p pr (r c) -> p pr r c", c=16)
            # row memsets are contiguous (cheap on gpsimd); the single-element
            # strided column memsets cost ~5us each there and would delay the
            # gpsimd replication DMAs -- run those on the early-idle DVE
            nc.gpsimd.memset(a1r[:, :, 0, :], 0)
            nc.gpsimd.memset(a1r[:, :, 15, :], 0)
            nc.vector.memset(a1r[:, :, 1:15, 0], 0)
            nc.vector.memset(a1r[:, :, 1:15, 15], 0)
            # s-major so pair is the contiguous axis (needed by the a2t DMA);
            # four separate 16-pair tiles so each scatter wave's reads carry
            # no false dependency on later chunks' pool writes
            A2WAVES = [(0, 16, 7), (16, 16, 15), (32, 16, 23),
                       (48, 12, 29), (60, 4, 31)]
            a2b = []
            for w, (p0, np_, cc) in enumerate(A2WAVES):
                a2bw = apool.tile([128, 50, np_], F8, tag=f"a2b{w}",
                                  name=f"a2b{w}")
                a2b.append(a2bw)
                nc.gpsimd.memset(a2bw[:, 49, :], 0)

            # replication direct from DRAM in three waves (SBUF->SBUF
            # two-hop measured slower in this environment); waves 0-1 use all
            # three queues, wave 2 sync-only so scalar/gpsimd are clear when
            # sign/pool work starts
            # a small first wave gets conv1 started ~10us earlier (wave-0
            # completion is what gates the first real matmul at the ~125 MB/s
            # DMA floor); later waves have plenty of slack before their pairs
            # are consumed
            for wv, (p0, p1) in enumerate(((0, 8), (8, 16), (16, 32),
                                           (32, 64))):
                engs = ([nc.sync, nc.scalar, nc.gpsimd] if wv < 3
                        else [nc.sync])
                for t, (dy, dx) in enumerate(TAPS):
                    sh = dy * 30 + dx
                    dst = xv54[:, t, p0:p1, :]
                    src = xp6[:, p0:p1, sh:sh + 840]
                    engs[t % len(engs)].dma_start(dst, src)

            # remaining small consts: none is needed before ~33us (first
            # sign / first conv2 chunk / fc2), so they queue BEHIND the
            # replication waves instead of delaying wave 0 on scalar
            w2t = consts.tile([128, 9, 128], F8)
            nc.scalar.dma_start(w2t[:], w2s[:])
            m2t = consts.tile([128, 1], F32)
            nc.scalar.dma_start(m2t[:], m2v[:])
            idt = consts.tile([128, 128], BF16)
            nc.scalar.dma_start(idt[:], ident[:])

            # bulk fc weights: own pool entered after the x6 staging pool
            # exits, so the allocator can reuse that region; the sync-queue
            # FIFO still puts the transfers behind the replication loads.
            wpool_cm = tc.tile_pool(name=f"wfc1p{_rep}", bufs=1)
            wpool = wpool_cm.__enter__()
            wfc1t = wpool.tile([128, 25, 2048], F8)
            for c5 in range(5):
                nc.sync.dma_start(wfc1t[:, 5 * c5:5 * c5 + 5, :],
                                  wfc1[:, 5 * c5:5 * c5 + 5, :])
            # -thr3 as a single row: folded into the fc1 PSUM accumulation
            # via a K=1 matmul (ones^T x (-thr3)) instead of a 1 MB broadcast
            thr3r = consts.tile([1, 2048], F32, tag="thr3r")
            nc.sync.dma_start(thr3r[:], thr3[:])
            ones1 = consts.tile([1, 128], F32, tag="ones1")
            nc.vector.memset(ones1[:], 1.0)
            wfc2t = consts.tile([128, 16, 10], BF16, tag="wfc2t")
            nc.sync.dma_start(wfc2t[:], wfc2[:])

            # resident k-major activations for fc1: [128=(s*64+ci), c, b']
            a2t = apool.tile([128, 25, BPC], F8)

            a1ap = a1pad[:]
            a1tens = a1ap.tensor
            a1base = a1ap.offset

            def conv1_pair(cps1, pr):
                # conv1 one pair: 2 matmuls (halves) -> sign (ACT, the only
                # engine pairing a PSUM read with the bias add) -> vertical
                # max (DVE, fp8) -> horizontal max (GpSimd) into a1pad
                ps = cps1.tile([128, 2, 512], F32)
                psap = ps[:]
                xv = xpt[:, pr, :].rearrange("p (y c) -> p y c", c=30)
                for h in range(2):
                    # out AP permutes rows: even rows land in [0:196], odd in
                    # [196:392], so the vertical max reads two fully
                    # contiguous halves (DVE 2x packed mode)
                    mout = AP(psap.tensor, psap.offset + 512 * h,
                              [[1024, 128], [28, 7], [196, 2], [1, 28]])
                    nc.tensor.matmul(
                        mout, w1t[:],
                        xv[:, 14 * h:14 * h + 14, 0:28],
                        start=True, stop=True)
                a1f = s1pool.tile([128, 2, 392], BF16)
                nc.scalar.sign(a1f[:], ps[:, :, 0:392], bias=t1t[:])
                vt = vpool.tile([128, 2, 196], BF16)
                nc.vector.tensor_max(vt[:], a1f[:, :, 0:196],
                                     a1f[:, :, 196:392])
                vv = vt[:].rearrange("p h (yo xo two) -> p h yo xo two",
                                     two=2, xo=14)
                av = a1pad[:, pr, :].rearrange("p (r c) -> p r c", c=16)
                dst = av[:, 1:15, 1:15].rearrange("p (h yo) xo -> p h yo xo",
                                                  h=2)
                nc.vector.tensor_max(dst, vv[:, :, :, :, 0],
                                     vv[:, :, :, :, 1])

            def conv2_chunk(cps2, c):
                # conv2 chunk = 2 pairs: 9 taps accumulate in PSUM. Normal
                # (non-DoubleRow) fp8 matmuls: per-pair DoubleRow halves the
                # stream time but pays an un-shared 256-column LDWEIGHTS per
                # matmul, which measures slower on HW.
                ps = cps2.tile([128, 2, 14, 14], F32)
                base = a1pad[:, 2 * c:2 * c + 2, :].rearrange(
                    "p pr (r c) -> p pr r c", c=16)
                for ti, (dy, dx) in enumerate(TAPS):
                    nc.tensor.matmul(ps[:], w2t[:, ti, :],
                                     base[:, :, dy:dy + 14, dx:dx + 14],
                                     start=(ti == 0), stop=(ti == 8))
                # sign(p - m2) -> +-1 on ACT (the DVE is the saturated
                # engine in the conv phase; ACT has slack). a2 in +-1 makes
                # fc1's PSUM the true +-1 sum, so thr3 is plain m3eff and the
                # 0/1 K1 correction disappears host-side.
                a2s = s2pool.tile([128, 2, 14, 14], BF16)
                nc.scalar.sign(a2s[:], ps[:], bias=m2t[:])
                # maxpool of +-1: vertical then horizontal (DVE)
                a2v = a2s[:].rearrange("p pr (yo two) x -> p pr yo two x",
                                       two=2)
                vt2 = vpool.tile([128, 2, 7, 14], BF16, tag="vt2")
                nc.vector.tensor_max(vt2[:], a2v[:, :, :, 0, :],
                                     a2v[:, :, :, 1, :])
                vv2 = vt2[:].rearrange("p pr yo (xo two) -> p pr yo xo two",
                                       two=2)
                w = next(i for i, (p0, np_, cc) in enumerate(A2WAVES)
                         if p0 <= 2 * c < p0 + np_)
                wp0, wnp, wcc = A2WAVES[w]
                col = 2 * c - wp0
                dst2 = a2b[w][:, 0:49, col:col + 2].rearrange(
                    "p (yo xo) pr -> p pr yo xo", xo=7)
                nc.vector.tensor_max(dst2, vv2[:, :, :, :, 0],
                                     vv2[:, :, :, :, 1])
                # a2 -> a2t partition-scatter waves, on the sync queue while
                # conv is live (scalar queue must stay clear for ACT signs);
                # the tiny final wave fans out over all three queues
                if c == wcc:
                    a2wv = a2b[w][:].rearrange("p (c s) r -> p s c r", s=2)
                    # early waves ride the idle gpsimd queue (sync is
                    # backlogged behind the fc1 weight transfers); the last
                    # two waves land after those transfers drain and after
                    # scalar's final sign, so the faster HWDGE queues take
                    # them to shrink the pre-fc1 gap
                    if w <= 2:
                        engs = [nc.gpsimd] * 4
                    elif w == 3:
                        engs = [nc.sync, nc.gpsimd, nc.sync, nc.gpsimd]
                    else:
                        engs = [nc.scalar, nc.sync, nc.scalar, nc.sync]
                    for g in range(2):
                        for s in range(2):
                            src = a2wv[64 * g:64 * g + 64, s, :, :]
                            dstp = a2t[64 * s:64 * s + 64, :,
                                       64 * g + wp0:64 * g + wp0 + wnp]
                            engs[2 * g + s].dma_start(dstp, src)

            # ------------------------------------------------------------------
            # conv1 + conv2, emission-interleaved so the PE alternates between
            # them (PE queue is FIFO in emission order); conv2 chunk c trails
            # conv1 pairs (2c, 2c+1) by one step.
            # ------------------------------------------------------------------
            SKEW = 2   # conv2 chunk c runs SKEW steps after conv1 pairs 2c,2c+1
            with tc.tile_pool(name=f"cps1{_rep}", bufs=3, space="PSUM") as cps1, \
                 tc.tile_pool(name=f"cps2{_rep}", bufs=2, space="PSUM") as cps2:
                for s in range(32 + SKEW):
                    if s < 32:
                        conv1_pair(cps1, 2 * s)
                        conv1_pair(cps1, 2 * s + 1)
                    if s >= SKEW:
                        conv2_chunk(cps2, s - SKEW)

            # ------------------------------------------------------------------
            # fc1 (resident a2t, DoubleRow over k-chunk pairs) with fc2 fused
            # in: each 512-neuron bank finishes early, is thresholded (DVE),
            # PE-transposed and fed to the fc2 accumulation while the next
            # bank's fc1 matmuls run.
            # ------------------------------------------------------------------
            a3 = apool.tile([128, 2048], BF16)
            with tc.tile_pool(name=f"fps{_rep}", bufs=1, space="PSUM") as fps, \
                 tc.tile_pool(name=f"ops{_rep}", bufs=1, space="PSUM") as ops_, \
                 tc.tile_pool(name=f"tps{_rep}", bufs=2, space="PSUM") as tps, \
                 tc.tile_pool(name=f"a3t{_rep}", bufs=2) as a3tp:
                psf = fps.tile([128, 2048], F32)
                pso = ops_.tile([128, 10], F32)

                # cp-outer: each a2t k-chunk's LDWEIGHTS is reused across the
                # 4 output banks (13 loads instead of 52)
                for cp in range(12):
                    kt = a2t[:, 2 * cp:2 * cp + 2, :]
                    for oc in range(4):
                        nc.tensor.matmul(
                            psf[:, 512 * oc:512 * oc + 512], kt,
                            wfc1t[:, 2 * cp:2 * cp + 2,
                                  512 * oc:512 * oc + 512],
                            start=(cp == 0), stop=False,
                            perf_mode=mybir.MatmulPerfMode.DoubleRow)
                for oc in range(4):
                    nc.tensor.matmul(psf[:, 512 * oc:512 * oc + 512],
                                     a2t[:, 24, :],
                                     wfc1t[:, 24, 512 * oc:512 * oc + 512],
                                     start=False, stop=False)
                # K=1 fp32 matmul adds -thr3[o] to every image row, so the
                # activation threshold becomes a compare against zero
                for oc in range(4):
                    nc.tensor.matmul(psf[:, 512 * oc:512 * oc + 512],
                                     ones1[:],
                                     thr3r[:, 512 * oc:512 * oc + 512],
                                     start=False, stop=True)

                # threshold per bank (DVE), then PE-transpose + fc2 matmuls;
                # bank oc+1's threshold runs while bank oc's transposes do
                for oc in range(4):
                    nc.vector.tensor_scalar(
                        a3[:, 512 * oc:512 * oc + 512],
                        psf[:, 512 * oc:512 * oc + 512],
                        0.0, None, mybir.AluOpType.is_ge)
                    for ch in range(4 * oc, 4 * oc + 4):
                        tp = tps.tile([128, 128], BF16)
                        nc.tensor.transpose(
                            tp[:], a3[:, 128 * ch:128 * ch + 128], idt[:])
                        at = a3tp.tile([128, 128], BF16)
                        if ch % 2 == 0:
                            nc.vector.tensor_copy(at[:], tp[:])
                        else:
                            nc.scalar.copy(at[:], tp[:])
                        nc.tensor.matmul(pso[:], at[:], wfc2t[:, ch, :],
                                         start=(ch == 0), stop=(ch == 15))

                outt = consts.tile([BPC, 10], F32, tag="outt")
                nc.scalar.copy(outt[:], pso[:])
                nc.sync.dma_start(outp[:], outt[:])
            wpool_cm.__exit__(None, None, None)
            xpool_cm.__exit__(None, None, None)


# ---------------------------------------------------------------------------
# Host-side prep
# ---------------------------------------------------------------------------

def _binarize(w):
    return np.where(np.asarray(w, np.float32) >= 0, 1.0, -1.0).astype(np.float32)


def _prep(x, w1, b1, g1, be1, m1, v1, w2, b2, g2, be2, m2, v2,
          wfc1, bfc1, g3, be3, m3, v3, wfc2, bfc2, scale):
    B = x.shape[0]
    w1b = _binarize(w1)          # [64,1,3,3]
    w2b = _binarize(w2)          # [64,64,3,3]
    wfc1b = _binarize(wfc1)      # [2048,3136]
    wfc2b = _binarize(wfc2)      # [10,2048]

    s1 = np.asarray(g1, np.float32) / np.sqrt(np.asarray(v1, np.float32) + EPS)
    s2 = np.asarray(g2, np.float32) / np.sqrt(np.asarray(v2, np.float32) + EPS)
    s3 = np.asarray(g3, np.float32) / np.sqrt(np.asarray(v3, np.float32) + EPS)
    # sign(h + t1) == sign(bn1(h)) since s1 > 0 (conv bias b1 folded in)
    t1 = (np.asarray(be1, np.float32) / s1 - np.asarray(m1, np.float32)
          + np.asarray(b1, np.float32)).astype(np.float32)
    # block2 threshold: p >= m2eff  (p = integer conv2 psum)
    m2eff = (np.asarray(m2, np.float32) - np.asarray(b2, np.float32)
             - np.asarray(be2, np.float32) / s2).astype(np.float32)
    # fc1 on 0/1 inputs: n_pm = 2*n01 - K1; condition n_pm >= m3eff
    m3eff = (np.asarray(m3, np.float32) - np.asarray(bfc1, np.float32)
             - np.asarray(be3, np.float32) / s3).astype(np.float32)
    thr3 = m3eff                                       # a2 is +-1: no K1 term
    c2 = wfc2b.sum(axis=1).astype(np.float32)          # [10]

    # --- exact bf16x3 split of the padded input ---
    xs = np.asarray(x, np.float32).reshape(B, 28, 28)
    xpad = np.zeros((B, 30, 30), np.float32)
    xpad[:, 1:29, 1:29] = xs
    xh = xpad.astype(NP_BF16)
    r = xpad - xh.astype(np.float32)
    xm = r.astype(NP_BF16)
    xl = (r - xm.astype(np.float32)).astype(NP_BF16)
    planes = np.stack([xh, xm, xl])                    # [3, B, 30, 30] bf16
    planes = planes.reshape(3, B, 900)
    # compact device image: row p6 = g*3 + l holds the unshifted padded plane
    # of parity-g images; the 9 tap shifts are applied by on-chip DMAs
    xp6_all = np.zeros((6, B // 2, 904), NP_BF16)
    for g in range(2):
        for l in range(3):
            xp6_all[g * 3 + l, :, 0:900] = planes[l, g::2, :]

    # conv1 stationary weights [54, 128] (block-diagonal over parity)
    w1sa = np.zeros((54, 128), np.float32)
    for g in range(2):
        for l in range(3):
            for ti, (dy, dx) in enumerate(TAPS):
                w1sa[g * 27 + l * 9 + ti, g * 64:g * 64 + 64] = w1b[:, 0, dy, dx]
    w1sa = w1sa.astype(NP_BF16)

    # conv2 weights [128, 9, 128] (partition-major) block-diag over parity
    w2sa = np.zeros((9, 128, 128), np.float32)
    for ti, (dy, dx) in enumerate(TAPS):
        blk = w2b[:, :, dy, dx].T                      # [ci, co]
        w2sa[ti, 0:64, 0:64] = blk
        w2sa[ti, 64:128, 64:128] = blk
    w2sa = np.ascontiguousarray(w2sa.transpose(1, 0, 2).astype(NP_F8))

    # fc1 weights [128, 25, 2048]: row (64*s + ci) of chunk c = feature
    # (ci, pos=2c+s); pos 49 (chunk 24, s=1) is zero padding
    Wp = np.zeros((2048, 64, 50), np.float32)
    Wp[:, :, :49] = wfc1b.reshape(2048, 64, 49)
    # [o, ci, c, s] -> [s, ci, c, o]
    wfc1r = Wp.reshape(2048, 64, 25, 2).transpose(3, 1, 2, 0).reshape(
        128, 25, 2048).astype(NP_F8)
    wfc1r = np.ascontiguousarray(wfc1r)

    # fc2 weights [128, 16, 10] (partition-major)
    wfc2r = np.ascontiguousarray(
        wfc2b.T.reshape(16, 128, 10).transpose(1, 0, 2).astype(NP_BF16))

    t1vv = np.concatenate([t1, t1]).reshape(128, 1).astype(np.float32)
    m2vv = np.concatenate([-m2eff, -m2eff]).reshape(128, 1).astype(np.float32)
    thr3n = np.ascontiguousarray((-thr3).reshape(1, 2048).astype(np.float32))
    identity = np.eye(128, dtype=NP_BF16)

    in_maps = []
    for c in range(N_CORES):
        xp_c = np.ascontiguousarray(xp6_all[:, c * PAIRS:(c + 1) * PAIRS, :])
        in_maps.append({
            "xp6": xp_c,
            "w1s": w1sa,
            "w2s": w2sa,
            "wfc1r": wfc1r,
            "wfc2r": wfc2r,
            "t1v": t1vv,
            "m2v": m2vv,
            "thr3n": thr3n,
            "ident": identity,
        })
    fixup = (c2, np.asarray(bfc2, np.float32), np.float32(np.asarray(scale)))
    return in_maps, fixup


# ---------------------------------------------------------------------------
# Cached compiled program + runner
# ---------------------------------------------------------------------------

_STATE = {}


def _get_runner():
    if "runner" in _STATE:
        return _STATE["runner"]
    nc = _build_nc()
    _STATE["nc"] = nc

    from concourse import bass2jax
    import jax

    bass2jax.install_neuronx_cc_hook()

    partition_name = (nc.partition_id_tensor.name
                      if nc.partition_id_tensor else None)
    in_names = []
    out_names = []
    out_avals = []
    zero_shapes = []
    for alloc in nc.m.functions[0].allocations:
        if not isinstance(alloc, mybir.MemoryLocationSet):
            continue
        name = alloc.memorylocations[0].name
        if alloc.kind == "ExternalInput":
            if name != partition_name:
                in_names.append(name)
        elif alloc.kind == "ExternalOutput":
            shape = tuple(alloc.tensor_shape)
            dtype = mybir.dt.np(alloc.dtype)
            out_names.append(name)
            out_avals.append(jax.core.ShapedArray(shape, dtype))
            zero_shapes.append((shape, dtype))
    n_params = len(in_names)
    n_outs = len(out_names)
    all_in_names = in_names + out_names
    if partition_name is not None:
        all_in_names = all_in_names + [partition_name]

    def _bodyfn(*args):
        operands = list(args)
        if partition_name is not None:
            operands.append(bass2jax.partition_id_tensor())
        outs = bass2jax._bass_exec_p.bind(
            *operands,
            out_avals=tuple(out_avals),
            in_names=tuple(all_in_names),
            out_names=tuple(out_names),
            lowering_input_output_aliases=(),
            sim_require_finite=True,
            sim_require_nnan=True,
            nc=nc,
        )
        return tuple(outs)

    from jax.sharding import Mesh, PartitionSpec
    from jax.experimental.shard_map import shard_map

    devices = jax.devices()[:N_CORES]
    mesh = Mesh(np.asarray(devices), ("core",))
    in_specs = (PartitionSpec("core"),) * (n_params + n_outs)
    out_specs = (PartitionSpec("core"),) * n_outs
    donate = tuple(range(n_params, n_params + n_outs))
    sharded = jax.jit(
        shard_map(_bodyfn, mesh=mesh, in_specs=in_specs,
                  out_specs=out_specs, check_rep=False),
        donate_argnums=donate, keep_unused=True)

    _STATE.update(dict(
        mesh=mesh, in_specs=in_specs, out_specs=out_specs,
        bodyfn=_bodyfn, in_names=in_names, zero_shapes=zero_shapes,
        n_params=n_params))

    def run(in_maps):
        per_core = [[np.asarray(m[nm]) for nm in in_names] for m in in_maps]
        concat_in = [
            np.concatenate([per_core[c][i] for c in range(N_CORES)], axis=0)
            for i in range(n_params)
        ]
        concat_zeros = [
            np.zeros((N_CORES * s[0], *s[1:]), d) for (s, d) in zero_shapes
        ]
        out_arrs = sharded(*concat_in, *concat_zeros)
        res = np.asarray(out_arrs[0]).reshape(N_CORES, BPC, 10)
        # device rows are ordered b' = g*64 + pair; restore b = 2*pair + g
        res = res.reshape(N_CORES, 2, PAIRS, 10).transpose(0, 2, 1, 3)
        return res.reshape(N_CORES, BPC, 10)

    _STATE["runner"] = run
    return run


def kernel(**inputs):
    in_maps, (c2, bfc2, scale) = _prep(**inputs)
    run = _get_runner()
    J = run(in_maps)                                   # [8, 128, 10] fp32
    J = J.reshape(N_CORES * BPC, 10)
    # exact integer fixup: h3@W = 2*J - c2 ; out = (I + bfc2) * scale in fp32
    I = (2.0 * J.astype(np.float64) - c2.astype(np.float64)).astype(np.float32)
    out = (I + bfc2[None, :]) * scale
    return out.astype(np.float32)


# expose in_maps/nc for the test harness (profiling path)
def _debug_handles(inputs):
    in_maps, fixup = _prep(**inputs)
    nc = _STATE.get("nc")
    if nc is None:
        _get_runner()
        nc = _STATE["nc"]
    return nc, in_maps, fixup


def _timed_exec(in_maps, iters=32):
    """Measure per-execution device time by queueing `iters` async
    executions of the NEFF with device-resident inputs (non-donating jit,
    so all buffers stay put) and timing tail-to-tail."""
    import time
    import jax
    from jax.experimental.shard_map import shard_map

    _get_runner()
    mesh = _STATE["mesh"]
    in_names = _STATE["in_names"]
    zero_shapes = _STATE["zero_shapes"]
    n_params = _STATE["n_params"]
    from jax.sharding import NamedSharding, PartitionSpec

    fn = jax.jit(
        shard_map(_STATE["bodyfn"], mesh=mesh, in_specs=_STATE["in_specs"],
                  out_specs=_STATE["out_specs"], check_rep=False),
        keep_unused=True)

    per_core = [[np.asarray(m[nm]) for nm in in_names] for m in in_maps]
    concat_in = [
        np.concatenate([per_core[c][i] for c in range(N_CORES)], axis=0)
        for i in range(n_params)
    ]
    concat_zeros = [
        np.zeros((N_CORES * s[0], *s[1:]), d) for (s, d) in zero_shapes
    ]
    sh = NamedSharding(mesh, PartitionSpec("core"))
    dev_in = [jax.device_put(a, sh) for a in concat_in]
    dev_zero = [jax.device_put(a, sh) for a in concat_zeros]

    out = fn(*dev_in, *dev_zero)
    jax.block_until_ready(out)
    # warm pass then timed async batches
    best = float("inf")
    for _ in range(3):
        t0 = time.perf_counter()
        outs = [fn(*dev_in, *dev_zero) for _ in range(iters)]
        jax.block_until_ready(outs)
        t1 = time.perf_counter()
        best = min(best, (t1 - t0) / iters)
    return best


# revision 51
# speedup vs baseline: 1.1772x; 1.1772x over previous
"""BNN-MNIST forward pass as a hand-written Bass/Tile kernel, data-parallel
across 8 TRN2 NeuronCores (batch 1024 -> 128 images per core).

Numerical scheme (everything except conv1 is EXACT vs the fp32 reference):
  - conv1: weights are binarized (+-1, exact in bf16); x is split exactly into
    3 bf16 planes (x = hi + mid + lo). The 3x3 conv over 1 input channel is a
    single matmul with contraction = (parity 2, level 3, tap 9) = 54 rows and
    lhs free = (parity 2, out-channel 64) = 128 (block-diagonal weights), so
    two images are computed per streamed column. Products are exact; only the
    PE fp32 accumulation order differs from the CPU reference (ulp-level).
    The 9 tap-shifted copies of each (parity, level) plane are produced
    ON-CHIP: each 32-pair half of the compact [6, 64, 904] DRAM image is
    loaded once with big contiguous descriptors, then 9 SBUF->SBUF
    replication DMAs apply the tap shifts (partition-strided dst) --
    avoiding 5.8 MB of re-reads through the slow HBM DMA path. conv1
    matmuls write even/odd rows to split PSUM halves via a permuted out-AP.
  - sign1 (bn+clip+binarize folded to sign(h + t1)): ACT Sign, +-1 bf16;
    maxpool1 on DVE (vertical max reads the two contiguous row-halves,
    horizontal max strided) into fp8 a1pad; conv1 and conv2 are
    emission-interleaved so the PE alternates between them.
  - conv2: +-1 weights x +-1 activations in fp8, 9 taps accumulate in PSUM
    (exact integers). Contraction = (parity 2, in-channel 64) block-diag.
  - sign2: step(p - m2) per element on DVE (0/1, exact integer compare),
    maxpool2 on DVE.
  - fc1: 0/1 activations vs +-1 weights in fp8, exact integer PSUM; the 0/1
    correction and bn3 threshold fold into thr3[o] = (m3' + K1[o])/2, which
    is subtracted inside the PSUM accumulation by a K=1 matmul
    (ones^T x -thr3) so the activation is a compare against zero. The
    a2 -> k-major transpose is done with SBUF->SBUF partition-scatter DMA
    waves (no DRAM round trip); fc1 matmuls read the resident tile directly.
  - fc2: 0/1 activations, exact integer result J; a3 chunks are transposed on
    the PE (identity-matmul transpose mode) instead of DMA-transposes. Host
    computes the exact affine fixup out = (2J - sum(wfc2b) + bfc2) * scale.
  - A burst of warmup matmuls on a zeroed tile keeps the PE busy from t=0 so
    the HAM clock-gate lifts to 2.4 GHz before conv1's first real matmul.
"""

import functools
import numpy as np
import ml_dtypes

import concourse.bass as bass
import concourse.tile as tile
from concourse import bacc, mybir
from concourse.ap import AP
from concourse.bass_utils import run_bass_kernel_spmd

F32 = mybir.dt.float32
BF16 = mybir.dt.bfloat16
F8 = mybir.dt.float8e4

NP_BF16 = ml_dtypes.bfloat16
NP_F8 = ml_dtypes.float8_e4m3

EPS = 1e-5
N_CORES = 8
BPC = 128          # images per core
PAIRS = BPC // 2   # image pairs per core
TAPS = [(dy, dx) for dy in range(3) for dx in range(3)]
TAP_PAIRS = [(0, 1), (2, 3), (4, 5), (6, 7)]   # DoubleRow tap pairs; tap 8 single
N_WARMUP = 48      # warmup matmuls (N=512) to lift HAM before conv1


# ---------------------------------------------------------------------------
# Device kernel builder
# ---------------------------------------------------------------------------

def _build_nc(reps=1):
    nc = bacc.Bacc("TRN2", target_bir_lowering=False, debug=False,
                   num_devices=N_CORES)

    xp6 = nc.declare_dram_parameter("xp6", [6, PAIRS, 904], BF16, isOutput=False)
    w1s = nc.declare_dram_parameter("w1s", [54, 128], BF16, isOutput=False)
    w2s = nc.declare_dram_parameter("w2s", [128, 9, 128], F8, isOutput=False)
    wfc1 = nc.declare_dram_parameter("wfc1r", [128, 25, 2048], F8, isOutput=False)
    wfc2 = nc.declare_dram_parameter("wfc2r", [128, 16, 10], BF16, isOutput=False)
    t1v = nc.declare_dram_parameter("t1v", [128, 1], F32, isOutput=False)
    m2v = nc.declare_dram_parameter("m2v", [128, 1], F32, isOutput=False)
    thr3 = nc.declare_dram_parameter("thr3n", [1, 2048], F32, isOutput=False)
    ident = nc.declare_dram_parameter("ident", [128, 128], BF16, isOutput=False)
    outp = nc.declare_dram_parameter("out", [BPC, 10], F32, isOutput=True)

    with tile.TileContext(nc) as tc:
        _body(nc, tc, xp6, w1s, w2s, wfc1, wfc2, t1v, m2v, thr3, ident, outp,
              reps=reps)

    nc.compile()
    return nc


def _body(nc, tc, xp6, w1s, w2s, wfc1, wfc2, t1v, m2v, thr3, ident, outp,
          reps=1):
    from contextlib import ExitStack
    with ExitStack() as ctx:
        consts = ctx.enter_context(tc.tile_pool(name="consts", bufs=1))
        apool = ctx.enter_context(tc.tile_pool(name="acts", bufs=1))
        s1pool = ctx.enter_context(tc.tile_pool(name="s1", bufs=2))
        vpool = ctx.enter_context(tc.tile_pool(name="vt", bufs=2))
        s2pool = ctx.enter_context(tc.tile_pool(name="s2", bufs=3))

        for _rep in range(reps):
            # ------------------------------------------------------------------
            # Warmup: PE matmuls on a zeroed tile from t=0 so the HAM clock
            # gate flips to 8/8 (~3.4us of sustained activity) while the
            # conv1 input replication DMAs are still in flight.
            # ------------------------------------------------------------------
            wz = consts.tile([128, 512], F8, tag="wz")
            nc.vector.memset(wz[:], 0)
            with tc.tile_pool(name=f"wps{_rep}", bufs=1, space="PSUM") as wpsp:
                wps = wpsp.tile([128, 512], F32)
                for _ in range(N_WARMUP):
                    nc.tensor.matmul(wps[:], wz[:, 0:128], wz[:],
                                     start=True, stop=True)

            # w1t (first conv1 matmul) and t1t (first sign) are needed
            # early and are tiny -- load them ahead of the replication waves
            w1t = consts.tile([54, 128], BF16)
            nc.sync.dma_start(w1t[:], w1s[:])
            t1t = consts.tile([128, 1], F32)
            nc.scalar.dma_start(t1t[:], t1v[:])

            # ------------------------------------------------------------------
            # conv1 input: on-chip 9-tap replication. xpt[p = g*27+l*9+t] holds
            # the (g,l) plane shifted by tap t. 18 DMAs (9 taps x 2 pair
            # halves) DRAM->SBUF, partition-strided dst, spread over 4 queues.
            # ------------------------------------------------------------------
            xpool_cm = tc.tile_pool(name=f"xplanes{_rep}", bufs=1)
            xpool = xpool_cm.__enter__()
            xpt = xpool.tile([54, PAIRS, 840], BF16)
            xv54 = xpt[:].rearrange("(gl t) pair e -> gl t pair e", t=9)
            a1pad = apool.tile([128, PAIRS, 256], F8)
            # zero only the padding ring of each 16x16 pair-plane
            a1r = a1pad[:].rearrange("p pr (r c) -> p pr r c", c=16)
            # row memsets are contiguous (cheap on gpsimd); the single-element
            # strided column memsets cost ~5us each there and would delay the
            # gpsimd replication DMAs -- run those on the early-idle DVE
            nc.gpsimd.memset(a1r[:, :, 0, :], 0)
            nc.gpsimd.memset(a1r[:, :, 15, :], 0)
            nc.vector.memset(a1r[:, :, 1:15, 0], 0)
            nc.vector.memset(a1r[:, :, 1:15, 15], 0)
            # s-major so pair is the contiguous axis (needed by the a2t DMA);
            # four separate 16-pair tiles so each scatter wave's reads carry
            # no false dependency on later chunks' pool writes
            A2WAVES = [(0, 16, 7), (16, 16, 15), (32, 16, 23),
                       (48, 12, 29), (60, 4, 31)]
            a2b = []
            for w, (p0, np_, cc) in enumerate(A2WAVES):
                a2bw = apool.tile([128, 50, np_], F8, tag=f"a2b{w}",
                                  name=f"a2b{w}")
                a2b.append(a2bw)
                nc.gpsimd.memset(a2bw[:, 49, :], 0)

            # replication direct from DRAM in three waves (SBUF->SBUF
            # two-hop measured slower in this environment); waves 0-1 use all
            # three queues, wave 2 sync-only so scalar/gpsimd are clear when
            # sign/pool work starts
            # three waves, 16/16/32 pairs: finer first waves were measured
            # SLOWER (the extra dma_start issue costs delay every later
            # wave's completion at the ~125 MB/s aggregate DMA floor)
            for wv, (p0, p1) in enumerate(((0, 16), (16, 32), (32, 64))):
                engs = [nc.sync, nc.scalar, nc.gpsimd] if wv < 2 else [nc.sync]
                for t, (dy, dx) in enumerate(TAPS):
                    sh = dy * 30 + dx
                    dst = xv54[:, t, p0:p1, :]
                    src = xp6[:, p0:p1, sh:sh + 840]
                    engs[t % len(engs)].dma_start(dst, src)

            # remaining small consts: none is needed before ~33us (first
            # sign / first conv2 chunk / fc2), so they queue BEHIND the
            # replication waves instead of delaying wave 0 on scalar
            w2t = consts.tile([128, 9, 128], F8)
            nc.scalar.dma_start(w2t[:], w2s[:])
            m2t = consts.tile([128, 1], F32)
            nc.scalar.dma_start(m2t[:], m2v[:])
            idt = consts.tile([128, 128], BF16)
            nc.scalar.dma_start(idt[:], ident[:])

            # bulk fc weights: own pool entered after the x6 staging pool
            # exits, so the allocator can reuse that region; the sync-queue
            # FIFO still puts the transfers behind the replication loads.
            wpool_cm = tc.tile_pool(name=f"wfc1p{_rep}", bufs=1)
            wpool = wpool_cm.__enter__()
            wfc1t = wpool.tile([128, 25, 2048], F8)
            for c5 in range(5):
                nc.sync.dma_start(wfc1t[:, 5 * c5:5 * c5 + 5, :],
                                  wfc1[:, 5 * c5:5 * c5 + 5, :])
            # -thr3 as a single row: folded into the fc1 PSUM accumulation
            # via a K=1 matmul (ones^T x (-thr3)) instead of a 1 MB broadcast
            thr3r = consts.tile([1, 2048], F32, tag="thr3r")
            nc.sync.dma_start(thr3r[:], thr3[:])
            ones1 = consts.tile([1, 128], F32, tag="ones1")
            nc.vector.memset(ones1[:], 1.0)
            wfc2t = consts.tile([128, 16, 10], BF16, tag="wfc2t")
            nc.sync.dma_start(wfc2t[:], wfc2[:])

            # resident k-major activations for fc1: [128=(s*64+ci), c, b']
            a2t = apool.tile([128, 25, BPC], F8)

            a1ap = a1pad[:]
            a1tens = a1ap.tensor
            a1base = a1ap.offset

            def conv1_pair(cps1, pr):
                # conv1 one pair: 2 matmuls (halves) -> sign (ACT, the only
                # engine pairing a PSUM read with the bias add) -> vertical
                # max (DVE, fp8) -> horizontal max (GpSimd) into a1pad
                ps = cps1.tile([128, 2, 512], F32)
                psap = ps[:]
                xv = xpt[:, pr, :].rearrange("p (y c) -> p y c", c=30)
                for h in range(2):
                    # out AP permutes rows: even rows land in [0:196], odd in
                    # [196:392], so the vertical max reads two fully
                    # contiguous halves (DVE 2x packed mode)
                    mout = AP(psap.tensor, psap.offset + 512 * h,
                              [[1024, 128], [28, 7], [196, 2], [1, 28]])
                    nc.tensor.matmul(
                        mout, w1t[:],
                        xv[:, 14 * h:14 * h + 14, 0:28],
                        start=True, stop=True)
                a1f = s1pool.tile([128, 2, 392], BF16)
                nc.scalar.sign(a1f[:], ps[:, :, 0:392], bias=t1t[:])
                vt = vpool.tile([128, 2, 196], BF16)
                nc.vector.tensor_max(vt[:], a1f[:, :, 0:196],
                                     a1f[:, :, 196:392])
                vv = vt[:].rearrange("p h (yo xo two) -> p h yo xo two",
                                     two=2, xo=14)
                av = a1pad[:, pr, :].rearrange("p (r c) -> p r c", c=16)
                dst = av[:, 1:15, 1:15].rearrange("p (h yo) xo -> p h yo xo",
                                                  h=2)
                nc.vector.tensor_max(dst, vv[:, :, :, :, 0],
                                     vv[:, :, :, :, 1])

            def conv2_chunk(cps2, c):
                # conv2 chunk = 2 pairs: 9 taps accumulate in PSUM. Normal
                # (non-DoubleRow) fp8 matmuls: per-pair DoubleRow halves the
                # stream time but pays an un-shared 256-column LDWEIGHTS per
                # matmul, which measures slower on HW.
                ps = cps2.tile([128, 2, 14, 14], F32)
                base = a1pad[:, 2 * c:2 * c + 2, :].rearrange(
                    "p pr (r c) -> p pr r c", c=16)
                for ti, (dy, dx) in enumerate(TAPS):
                    nc.tensor.matmul(ps[:], w2t[:, ti, :],
                                     base[:, :, dy:dy + 14, dx:dx + 14],
                                     start=(ti == 0), stop=(ti == 8))
                # sign(p - m2) -> +-1 on ACT (the DVE is the saturated
                # engine in the conv phase; ACT has slack). a2 in +-1 makes
                # fc1's PSUM the true +-1 sum, so thr3 is plain m3eff and the
                # 0/1 K1 correction disappears host-side.
                a2s = s2pool.tile([128, 2, 14, 14], BF16)
                nc.scalar.sign(a2s[:], ps[:], bias=m2t[:])
                # maxpool of +-1: vertical then horizontal (DVE)
                a2v = a2s[:].rearrange("p pr (yo two) x -> p pr yo two x",
                                       two=2)
                vt2 = vpool.tile([128, 2, 7, 14], BF16, tag="vt2")
                nc.vector.tensor_max(vt2[:], a2v[:, :, :, 0, :],
                                     a2v[:, :, :, 1, :])
                vv2 = vt2[:].rearrange("p pr yo (xo two) -> p pr yo xo two",
                                       two=2)
                w = next(i for i, (p0, np_, cc) in enumerate(A2WAVES)
                         if p0 <= 2 * c < p0 + np_)
                wp0, wnp, wcc = A2WAVES[w]
                col = 2 * c - wp0
                dst2 = a2b[w][:, 0:49, col:col + 2].rearrange(
                    "p (yo xo) pr -> p pr yo xo", xo=7)
                nc.vector.tensor_max(dst2, vv2[:, :, :, :, 0],
                                     vv2[:, :, :, :, 1])
                # a2 -> a2t partition-scatter waves, on the sync queue while
                # conv is live (scalar queue must stay clear for ACT signs);
                # the tiny final wave fans out over all three queues
                if c == wcc:
                    a2wv = a2b[w][:].rearrange("p (c s) r -> p s c r", s=2)
                    # early waves ride the idle gpsimd queue (sync is
                    # backlogged behind the fc1 weight transfers); the last
                    # two waves land after those transfers drain and after
                    # scalar's final sign, so the faster HWDGE queues take
                    # them to shrink the pre-fc1 gap
                    if w <= 2:
                        engs = [nc.gpsimd] * 4
                    elif w == 3:
                        engs = [nc.sync, nc.gpsimd, nc.sync, nc.gpsimd]
                    else:
                        engs = [nc.scalar, nc.sync, nc.scalar, nc.sync]
                    for g in range(2):
                        for s in range(2):
                            src = a2wv[64 * g:64 * g + 64, s, :, :]
                            dstp = a2t[64 * s:64 * s + 64, :,
                                       64 * g + wp0:64 * g + wp0 + wnp]
                            engs[2 * g + s].dma_start(dstp, src)

            # ------------------------------------------------------------------
            # conv1 + conv2, emission-interleaved so the PE alternates between
            # them (PE queue is FIFO in emission order); conv2 chunk c trails
            # conv1 pairs (2c, 2c+1) by one step.
            # ------------------------------------------------------------------
            SKEW = 2   # conv2 chunk c runs SKEW steps after conv1 pairs 2c,2c+1
            with tc.tile_pool(name=f"cps1{_rep}", bufs=3, space="PSUM") as cps1, \
                 tc.tile_pool(name=f"cps2{_rep}", bufs=2, space="PSUM") as cps2:
                for s in range(32 + SKEW):
                    if s < 32:
                        conv1_pair(cps1, 2 * s)
                        conv1_pair(cps1, 2 * s + 1)
                    if s >= SKEW:
                        conv2_chunk(cps2, s - SKEW)

            # ------------------------------------------------------------------
            # fc1 (resident a2t, DoubleRow over k-chunk pairs) with fc2 fused
            # in: each 512-neuron bank finishes early, is thresholded (DVE),
            # PE-transposed and fed to the fc2 accumulation while the next
            # bank's fc1 matmuls run.
            # ------------------------------------------------------------------
            a3 = apool.tile([128, 2048], BF16)
            with tc.tile_pool(name=f"fps{_rep}", bufs=1, space="PSUM") as fps, \
                 tc.tile_pool(name=f"ops{_rep}", bufs=1, space="PSUM") as ops_, \
                 tc.tile_pool(name=f"tps{_rep}", bufs=2, space="PSUM") as tps, \
                 tc.tile_pool(name=f"a3t{_rep}", bufs=2) as a3tp:
                psf = fps.tile([128, 2048], F32)
                pso = ops_.tile([128, 10], F32)

                # cp-outer: each a2t k-chunk's LDWEIGHTS is reused across the
                # 4 output banks (13 loads instead of 52)
                for cp in range(12):
                    kt = a2t[:, 2 * cp:2 * cp + 2, :]
                    for oc in range(4):
                        nc.tensor.matmul(
                            psf[:, 512 * oc:512 * oc + 512], kt,
                            wfc1t[:, 2 * cp:2 * cp + 2,
                                  512 * oc:512 * oc + 512],
                            start=(cp == 0), stop=False,
                            perf_mode=mybir.MatmulPerfMode.DoubleRow)
                for oc in range(4):
                    nc.tensor.matmul(psf[:, 512 * oc:512 * oc + 512],
                                     a2t[:, 24, :],
                                     wfc1t[:, 24, 512 * oc:512 * oc + 512],
                                     start=False, stop=False)
                # K=1 fp32 matmul adds -thr3[o] to every image row, so the
                # activation threshold becomes a compare against zero
                for oc in range(4):
                    nc.tensor.matmul(psf[:, 512 * oc:512 * oc + 512],
                                     ones1[:],
                                     thr3r[:, 512 * oc:512 * oc + 512],
                                     start=False, stop=True)

                # threshold per bank (DVE), then PE-transpose + fc2 matmuls;
                # bank oc+1's threshold runs while bank oc's transposes do
                for oc in range(4):
                    nc.vector.tensor_scalar(
                        a3[:, 512 * oc:512 * oc + 512],
                        psf[:, 512 * oc:512 * oc + 512],
                        0.0, None, mybir.AluOpType.is_ge)
                    for ch in range(4 * oc, 4 * oc + 4):
                        tp = tps.tile([128, 128], BF16)
                        nc.tensor.transpose(
                            tp[:], a3[:, 128 * ch:128 * ch + 128], idt[:])
                        at = a3tp.tile([128, 128], BF16)
                        if ch % 2 == 0:
                            nc.vector.tensor_copy(at[:], tp[:])
                        else:
                            nc.scalar.copy(at[:], tp[:])
                        nc.tensor.matmul(pso[:], at[:], wfc2t[:, ch, :],
                                         start=(ch == 0), stop=(ch == 15))

                outt = consts.tile([BPC, 10], F32, tag="outt")
                nc.scalar.copy(outt[:], pso[:])
                nc.sync.dma_start(outp[:], outt[:])
            wpool_cm.__exit__(None, None, None)
            xpool_cm.__exit__(None, None, None)


# ---------------------------------------------------------------------------
# Host-side prep
# ---------------------------------------------------------------------------

def _binarize(w):
    return np.where(np.asarray(w, np.float32) >= 0, 1.0, -1.0).astype(np.float32)


def _prep(x, w1, b1, g1, be1, m1, v1, w2, b2, g2, be2, m2, v2,
          wfc1, bfc1, g3, be3, m3, v3, wfc2, bfc2, scale):
    B = x.shape[0]
    w1b = _binarize(w1)          # [64,1,3,3]
    w2b = _binarize(w2)          # [64,64,3,3]
    wfc1b = _binarize(wfc1)      # [2048,3136]
    wfc2b = _binarize(wfc2)      # [10,2048]

    s1 = np.asarray(g1, np.float32) / np.sqrt(np.asarray(v1, np.float32) + EPS)
    s2 = np.asarray(g2, np.float32) / np.sqrt(np.asarray(v2, np.float32) + EPS)
    s3 = np.asarray(g3, np.float32) / np.sqrt(np.asarray(v3, np.float32) + EPS)
    # sign(h + t1) == sign(bn1(h)) since s1 > 0 (conv bias b1 folded in)
    t1 = (np.asarray(be1, np.float32) / s1 - np.asarray(m1, np.float32)
          + np.asarray(b1, np.float32)).astype(np.float32)
    # block2 threshold: p >= m2eff  (p = integer conv2 psum)
    m2eff = (np.asarray(m2, np.float32) - np.asarray(b2, np.float32)
             - np.asarray(be2, np.float32) / s2).astype(np.float32)
    # fc1 on 0/1 inputs: n_pm = 2*n01 - K1; condition n_pm >= m3eff
    m3eff = (np.asarray(m3, np.float32) - np.asarray(bfc1, np.float32)
             - np.asarray(be3, np.float32) / s3).astype(np.float32)
    thr3 = m3eff                                       # a2 is +-1: no K1 term
    c2 = wfc2b.sum(axis=1).astype(np.float32)          # [10]

    # --- exact bf16x3 split of the padded input ---
    xs = np.asarray(x, np.float32).reshape(B, 28, 28)
    xpad = np.zeros((B, 30, 30), np.float32)
    xpad[:, 1:29, 1:29] = xs
    xh = xpad.astype(NP_BF16)
    r = xpad - xh.astype(np.float32)
    xm = r.astype(NP_BF16)
    xl = (r - xm.astype(np.float32)).astype(NP_BF16)
    planes = np.stack([xh, xm, xl])                    # [3, B, 30, 30] bf16
    planes = planes.reshape(3, B, 900)
    # compact device image: row p6 = g*3 + l holds the unshifted padded plane
    # of parity-g images; the 9 tap shifts are applied by on-chip DMAs
    xp6_all = np.zeros((6, B // 2, 904), NP_BF16)
    for g in range(2):
        for l in range(3):
            xp6_all[g * 3 + l, :, 0:900] = planes[l, g::2, :]

    # conv1 stationary weights [54, 128] (block-diagonal over parity)
    w1sa = np.zeros((54, 128), np.float32)
    for g in range(2):
        for l in range(3):
            for ti, (dy, dx) in enumerate(TAPS):
                w1sa[g * 27 + l * 9 + ti, g * 64:g * 64 + 64] = w1b[:, 0, dy, dx]
    w1sa = w1sa.astype(NP_BF16)

    # conv2 weights [128, 9, 128] (partition-major) block-diag over parity
    w2sa = np.zeros((9, 128, 128), np.float32)
    for ti, (dy, dx) in enumerate(TAPS):
        blk = w2b[:, :, dy, dx].T                      # [ci, co]
        w2sa[ti, 0:64, 0:64] = blk
        w2sa[ti, 64:128, 64:128] = blk
    w2sa = np.ascontiguousarray(w2sa.transpose(1, 0, 2).astype(NP_F8))

    # fc1 weights [128, 25, 2048]: row (64*s + ci) of chunk c = feature
    # (ci, pos=2c+s); pos 49 (chunk 24, s=1) is zero padding
    Wp = np.zeros((2048, 64, 50), np.float32)
    Wp[:, :, :49] = wfc1b.reshape(2048, 64, 49)
    # [o, ci, c, s] -> [s, ci, c, o]
    wfc1r = Wp.reshape(2048, 64, 25, 2).transpose(3, 1, 2, 0).reshape(
        128, 25, 2048).astype(NP_F8)
    wfc1r = np.ascontiguousarray(wfc1r)

    # fc2 weights [128, 16, 10] (partition-major)
    wfc2r = np.ascontiguousarray(
        wfc2b.T.reshape(16, 128, 10).transpose(1, 0, 2).astype(NP_BF16))

    t1vv = np.concatenate([t1, t1]).reshape(128, 1).astype(np.float32)
    m2vv = np.concatenate([-m2eff, -m2eff]).reshape(128, 1).astype(np.float32)
    thr3n = np.ascontiguousarray((-thr3).reshape(1, 2048).astype(np.float32))
    identity = np.eye(128, dtype=NP_BF16)

    in_maps = []
    for c in range(N_CORES):
        xp_c = np.ascontiguousarray(xp6_all[:, c * PAIRS:(c + 1) * PAIRS, :])
        in_maps.append({
            "xp6": xp_c,
            "w1s": w1sa,
            "w2s": w2sa,
            "wfc1r": wfc1r,
            "wfc2r": wfc2r,
            "t1v": t1vv,
            "m2v": m2vv,
            "thr3n": thr3n,
            "ident": identity,
        })
    fixup = (c2, np.asarray(bfc2, np.float32), np.float32(np.asarray(scale)))
    return in_maps, fixup


# ---------------------------------------------------------------------------
# Cached compiled program + runner
# ---------------------------------------------------------------------------

_STATE = {}


def _get_runner():
    if "runner" in _STATE:
        return _STATE["runner"]
    nc = _build_nc()
    _STATE["nc"] = nc

    from concourse import bass2jax
    import jax

    bass2jax.install_neuronx_cc_hook()

    partition_name = (nc.partition_id_tensor.name
                      if nc.partition_id_tensor else None)
    in_names = []
    out_names = []
    out_avals = []
    zero_shapes = []
    for alloc in nc.m.functions[0].allocations:
        if not isinstance(alloc, mybir.MemoryLocationSet):
            continue
        name = alloc.memorylocations[0].name
        if alloc.kind == "ExternalInput":
            if name != partition_name:
                in_names.append(name)
        elif alloc.kind == "ExternalOutput":
            shape = tuple(alloc.tensor_shape)
            dtype = mybir.dt.np(alloc.dtype)
            out_names.append(name)
            out_avals.append(jax.core.ShapedArray(shape, dtype))
            zero_shapes.append((shape, dtype))
    n_params = len(in_names)
    n_outs = len(out_names)
    all_in_names = in_names + out_names
    if partition_name is not None:
        all_in_names = all_in_names + [partition_name]

    def _bodyfn(*args):
        operands = list(args)
        if partition_name is not None:
            operands.append(bass2jax.partition_id_tensor())
        outs = bass2jax._bass_exec_p.bind(
            *operands,
            out_avals=tuple(out_avals),
            in_names=tuple(all_in_names),
            out_names=tuple(out_names),
            lowering_input_output_aliases=(),
            sim_require_finite=True,
            sim_require_nnan=True,
            nc=nc,
        )
        return tuple(outs)

    from jax.sharding import Mesh, PartitionSpec
    from jax.experimental.shard_map import shard_map

    devices = jax.devices()[:N_CORES]
    mesh = Mesh(np.asarray(devices), ("core",))
    in_specs = (PartitionSpec("core"),) * (n_params + n_outs)
    out_specs = (PartitionSpec("core"),) * n_outs
    donate = tuple(range(n_params, n_params + n_outs))
    sharded = jax.jit(
        shard_map(_bodyfn, mesh=mesh, in_specs=in_specs,
                  out_specs=out_specs, check_rep=False),
        donate_argnums=donate, keep_unused=True)

    _STATE.update(dict(
        mesh=mesh, in_specs=in_specs, out_specs=out_specs,
        bodyfn=_bodyfn, in_names=in_names, zero_shapes=zero_shapes,
        n_params=n_params))

    def run(in_maps):
        per_core = [[np.asarray(m[nm]) for nm in in_names] for m in in_maps]
        concat_in = [
            np.concatenate([per_core[c][i] for c in range(N_CORES)], axis=0)
            for i in range(n_params)
        ]
        concat_zeros = [
            np.zeros((N_CORES * s[0], *s[1:]), d) for (s, d) in zero_shapes
        ]
        out_arrs = sharded(*concat_in, *concat_zeros)
        res = np.asarray(out_arrs[0]).reshape(N_CORES, BPC, 10)
        # device rows are ordered b' = g*64 + pair; restore b = 2*pair + g
        res = res.reshape(N_CORES, 2, PAIRS, 10).transpose(0, 2, 1, 3)
        return res.reshape(N_CORES, BPC, 10)

    _STATE["runner"] = run
    return run


def kernel(**inputs):
    in_maps, (c2, bfc2, scale) = _prep(**inputs)
    run = _get_runner()
    J = run(in_maps)                                   # [8, 128, 10] fp32
    J = J.reshape(N_CORES * BPC, 10)
    # exact integer fixup: h3@W = 2*J - c2 ; out = (I + bfc2) * scale in fp32
    I = (2.0 * J.astype(np.float64) - c2.astype(np.float64)).astype(np.float32)
    out = (I + bfc2[None, :]) * scale
    return out.astype(np.float32)


# expose in_maps/nc for the test harness (profiling path)
def _debug_handles(inputs):
    in_maps, fixup = _prep(**inputs)
    nc = _STATE.get("nc")
    if nc is None:
        _get_runner()
        nc = _STATE["nc"]
    return nc, in_maps, fixup


def _timed_exec(in_maps, iters=32):
    """Measure per-execution device time by queueing `iters` async
    executions of the NEFF with device-resident inputs (non-donating jit,
    so all buffers stay put) and timing tail-to-tail."""
    import time
    import jax
    from jax.experimental.shard_map import shard_map

    _get_runner()
    mesh = _STATE["mesh"]
    in_names = _STATE["in_names"]
    zero_shapes = _STATE["zero_shapes"]
    n_params = _STATE["n_params"]
    from jax.sharding import NamedSharding, PartitionSpec

    fn = jax.jit(
        shard_map(_STATE["bodyfn"], mesh=mesh, in_specs=_STATE["in_specs"],
                  out_specs=_STATE["out_specs"], check_rep=False),
        keep_unused=True)

    per_core = [[np.asarray(m[nm]) for nm in in_names] for m in in_maps]
    concat_in = [
        np.concatenate([per_core[c][i] for c in range(N_CORES)], axis=0)
        for i in range(n_params)
    ]
    concat_zeros = [
        np.zeros((N_CORES * s[0], *s[1:]), d) for (s, d) in zero_shapes
    ]
    sh = NamedSharding(mesh, PartitionSpec("core"))
    dev_in = [jax.device_put(a, sh) for a in concat_in]
    dev_zero = [jax.device_put(a, sh) for a in concat_zeros]

    out = fn(*dev_in, *dev_zero)
    jax.block_until_ready(out)
    # warm pass then timed async batches
    best = float("inf")
    for _ in range(3):
        t0 = time.perf_counter()
        outs = [fn(*dev_in, *dev_zero) for _ in range(iters)]
        jax.block_until_ready(outs)
        t1 = time.perf_counter()
        best = min(best, (t1 - t0) / iters)
    return best
